# revision 2
# baseline (speedup 1.0000x reference)
"""CRF forward (logsumexp over paths) loss kernel for Trainium2, 8 NeuronCores.

Math
----
reference:  fv0 = alpha_0^T + emits[0]                       [B, K]
            fv_t[b,j] = logsumexp_i(fv_{t-1}[b,i] + trans[i,j]) + emit_t[b,j]
            alpha_z = sum_b logsumexp_k( fv_{tau_b}[b,:] )   (tau = one-hot mask step)

Exp-space recurrence: with ETs[i,j] = exp(trans[i,j] - DELTA) (plus a 65th
ones column) and e_t[k,b] = exp(emit_t[b,k]) transposed (plus a ones row),
the state w_t[k,b] = exp(fv_t[k,b] - DELTA*t) obeys

    w_t = (ETs^T w_{t-1}) * e_t        (one matmul + one elementwise mul)

No renormalization is needed: for DELTA = 5.125 the state stays within
~e^-20..e^+10 over all 512 steps (bf16 shares fp32's 8-bit exponent).  Row 64
of each matmul output is colsum(w_{t-1}) = Z_{t-1} (the logsumexp of fv_{t-1}
up to the known DELTA*t offset), captured for free through the ets ones
column and the emission ones row.

Time-parallel segmentation: the serial chain latency (~0.9us per step group:
PE->PSUM latency + DVE round trip) would force 512 x ~0.5us serial steps.
Products of positive matrices forget their initial condition (Birkhoff
contraction; mixing completes in <8 steps here), so time is cut into S=8
segments of 64 steps.  Segment s>=1 starts NMIX=8 steps early from an
all-ones state; after mixing its state equals the true state up to a
per-batch-column scalar.  The scalar is recovered by anchoring: both
segment s and segment s-1 compute Z at tau = 64s-1, and
log c_s = log Zg_s - log Z_{s-1} there.  Cascaded corrections are applied
via a segment-selector matmul in the final combine.

All 8 segments run concurrently in lockstep slots, grouped 4+4: per group
per slot ONE fused matmul (rhs = 4 segments' states, 256 cols) and ONE DVE
tensor_tensor [65,256] (amortizing the fixed ~125ns PSUM-access cost 4 ways).
Emissions are exp'ed on ACT into a PADDED natural-layout ring (128-col slot
blocks: 64 emission cols, a ones col at 64) and transposed by the DMA XBAR
engine (hardware 16x128-tile transpose, one DMA per segment-window) straight
into SBUF - the ones column lands as the ones ROW at partition 64, so the
tensor engine does nothing but the chain matmuls.  The full state history
stays in SBUF; colsum rows are DMA-captured per window into the Z table and
the mask-select reduction runs at the end.

Sharding: batch B=512 split across 8 cores (64 per core); transitions/alpha_0
replicated; final alpha_z = host sum of the 8 per-core [1,64] row outputs.
"""

import os
import sys

for _p in ("/opt/trn_rl_repo", "/root/.axon_site/_ro/trn_rl_repo"):
    if os.path.isdir(_p) and _p not in sys.path:
        sys.path.insert(0, _p)

from contextlib import ExitStack

import numpy as np

import concourse.bass as bass
import concourse.mybir as mybir
import concourse.tile as tile
from concourse.bass_utils import run_bass_kernel_spmd
from concourse.masks import make_identity

# The walrus build in this container rejects instructions carrying more than
# one sync-wait command ("Too many sync wait commands" in setupSyncWait).
# Tile freely emits multi-wait instructions, so split the extras onto
# preceding same-engine no-ops at commit time (engine queues execute
# in-order, so the semantics are identical).
_ORIG_COMMIT = tile.TileContext._commit_instruction


def _single_wait_commit(self, inst, lazy_reg_writes=True):
    si = getattr(inst, "sync_info", None)
    if (
        si is not None
        and si.on_wait
        and len(si.on_wait) > 1
        and inst.engine != mybir.EngineType.Unassigned
    ):
        waits = list(si.on_wait)
        eng = self.nc.engines[inst.engine]
        for w in waits[:-1]:
            n = eng.nop(nofuse=True)
            n.ins.sync_info = mybir.SyncInfo(on_wait=[w], on_update=[])
        inst.sync_info = mybir.SyncInfo(
            on_wait=[waits[-1]], on_update=list(si.on_update or [])
        )
    _ORIG_COMMIT(self, inst, lazy_reg_writes)


tile.TileContext._commit_instruction = _single_wait_commit

T, B, K = 512, 512, 64
NCORES = 8
BSH = B // NCORES          # 64 batch elements per core
S = 8                      # time segments
SEGLEN = T // S            # 64 output steps per segment
NMIX = 8                   # warm-up (mixing) steps for guessed segments
NSLOT = SEGLEN + NMIX      # 72 chain slots per segment
W = 8                      # slots per emission-staging window
NWINS = NSLOT // W         # 9 windows
GW = 4 * K                 # 256: group width (4 segments x 64 cols)
KP = K + 1                 # 65 rows: state + colsum row
ENB = K + 1                # 65: natural-layout slot block (emis | ones col)
ENRING = 3 * W             # 24-slot natural-emission ring
TRRING = 3 * W             # 24-slot transposed-emission ring
DELTA = 5.125              # per-step log-space offset folded into ETs
F32 = mybir.dt.float32
BF16 = mybir.dt.bfloat16
U8 = mybir.dt.uint8
I32 = mybir.dt.int32
MULT = mybir.AluOpType.mult
ADD = mybir.AluOpType.add
SUB = mybir.AluOpType.subtract
AX = mybir.AxisListType.X
AF = mybir.ActivationFunctionType


def _seg_t0(seg):  # t value of chain slot 0 (initial state)
    return 0 if seg == 0 else SEGLEN * seg - NMIX


def _last_em_slot(seg):  # last slot with a real emission
    if seg == 0:
        return SEGLEN          # seg 0 produces tau 0..63 from slots 1..64
    if seg == S - 1:
        return NSLOT - 1       # t(NSLOT) = 512 has no emission
    return NSLOT


def _build_crf_nc() -> bass.Bass:
    nc = bass.Bass(trn_type="TRN2", target_bir_lowering=False, debug=False)

    emits_d = nc.dram_tensor("emits", [T, BSH, K], F32, kind="ExternalInput").ap()
    mask_d = nc.dram_tensor("maskb", [T, BSH], U8, kind="ExternalInput").ap()
    trans_d = nc.dram_tensor("transitions", [K, K], F32, kind="ExternalInput").ap()
    alpha0_d = nc.dram_tensor("alpha_0", [K, 1], F32, kind="ExternalInput").ap()
    out_d = nc.dram_tensor("out_row", [1, BSH], F32, kind="ExternalOutput").ap()

    with tile.TileContext(nc) as tc:
        with ExitStack() as ctx:
            _crf_body(ctx, tc, emits_d, mask_d, trans_d, alpha0_d, out_d)
    _split_remaining_multiwaits(nc)
    return nc


def _split_remaining_multiwaits(nc):
    """Split multi-wait instructions added outside the commit path (e.g. the
    end-of-kernel drain/barrier) onto preceding same-engine no-ops."""
    for blk in nc.m.functions[0].blocks:
        il = blk.instructions
        idx = 0
        while idx < len(il):
            inst = il[idx]
            si = inst.sync_info
            if si is not None and si.on_wait and len(si.on_wait) > 1:
                waits = list(si.on_wait)
                for j, w in enumerate(waits[:-1]):
                    n = mybir.InstNoOp(
                        name=f"I-swx-{inst.name}-{j}", ins=[], outs=[]
                    )
                    n.engine = inst.engine
                    n.sync_info = mybir.SyncInfo(on_wait=[w], on_update=[])
                    nc.register_instruction(n, overwrite=True)
                    il.insert(idx, n)
                    idx += 1
                inst.sync_info = mybir.SyncInfo(
                    on_wait=[waits[-1]], on_update=list(si.on_update or [])
                )
            idx += 1


def _crf_body(ctx, tc, emits_d, mask_d, trans_d, alpha0_d, out_d):
    nc = tc.nc

    # ---- long-lived SBUF state ----
    ets = nc.alloc_sbuf_tensor("ets", [K, KP], BF16).ap()     # exp(trans-d)|1
    expal = nc.alloc_sbuf_tensor("expal", [KP, 1], F32).ap()  # exp(alpha_0)|1
    # per-group state history: slot sigma block = 4 segments x 64 cols,
    # row 64 of slot sigma = colsum of the slot sigma-1 state
    wh = [
        nc.alloc_sbuf_tensor(f"wh{g}", [KP, (NSLOT + 1) * GW], BF16).ap()
        for g in range(2)
    ]
    # transposed-emission rings: slot block = [65, 4 segs x 64], row 64 = ones
    etr = [
        nc.alloc_sbuf_tensor(f"etr{g}", [KP, TRRING * GW], BF16).ap()
        for g in range(2)
    ]
    # post-exp emissions, natural layout: slot block = [64, 65 (emis | ones)]
    en = [
        nc.alloc_sbuf_tensor(f"en{s}", [BSH, ENRING * ENB], BF16).ap()
        for s in range(S)
    ]
    ident = nc.alloc_sbuf_tensor("ident", [BSH, BSH], BF16).ap()
    maskw = nc.alloc_sbuf_tensor("maskw", [K, T], F32).ap()   # [win, tw*64+b]
    mk_u8 = nc.alloc_sbuf_tensor("mk_u8", [K, T], U8).ap()
    iota_i = nc.alloc_sbuf_tensor("iota_i", [K, T], I32).ap()
    iotaw = nc.alloc_sbuf_tensor("iotaw", [K, T], F32).ap()   # t at slot pos
    csum = nc.alloc_sbuf_tensor("csum", [K, T], BF16).ap()    # Z_tau table
    sel = nc.alloc_sbuf_tensor("sel", [K, S], F32).ap()       # win>=8r matrix
    ones_c = nc.alloc_sbuf_tensor("ones_c", [K, 1], F32).ap()
    ones8 = nc.alloc_sbuf_tensor("ones8", [S, 1], F32).ap()
    anch = nc.alloc_sbuf_tensor("anch", [S, BSH], BF16).ap()  # Zg_s anchors
    prev = nc.alloc_sbuf_tensor("prev", [S, BSH], BF16).ap()  # Z_{s-1} anchors
    cst = nc.alloc_sbuf_tensor("cst", [K, 2], F32).ap()       # bias constants

    # ---- pools ----
    em_pool = ctx.enter_context(tc.tile_pool(name="em", bufs=18))
    fin_pool = ctx.enter_context(tc.tile_pool(name="fin", bufs=1))
    # single shared pools with 3 bufs: allocation order alternates groups, so
    # the buffer-reuse dependency bounds group desync to ~1.5 slots and keeps
    # the static schedule honest
    st_shared = ctx.enter_context(tc.tile_pool(name="st", bufs=3, space="PSUM"))
    st_pool = [st_shared, st_shared]
    etp_shared = ctx.enter_context(tc.tile_pool(name="etp", bufs=3, space="PSUM"))
    etp_pool = [etp_shared, etp_shared]

    # ---- emission staging helpers ----
    def load_chunk(seg, w, eng=None):
        """DMA the raw fp32 emissions for (segment, window) into an em tile."""
        lo, hi = W * w + 1, min(W * w + W, _last_em_slot(seg))
        if lo > hi:
            return None
        t0 = _seg_t0(seg)
        cnt = hi - lo + 1
        em = em_pool.tile([BSH, W * K], F32, tag="em", name="em")
        (eng or nc.sync).dma_start(
            em[:, 0 : cnt * K].rearrange("b (t k) -> b t k", t=cnt),
            emits_d[t0 + lo : t0 + hi + 1].rearrange("t b k -> b t k"),
        )
        return em

    def exp_chunk(seg, w, em):
        """exp -> bf16 into the padded natural-layout emission ring."""
        lo, hi = W * w + 1, min(W * w + W, _last_em_slot(seg))
        if lo > hi:
            return
        cnt = hi - lo + 1
        p = (lo - 1) % ENRING
        dst = en[seg].rearrange("b (r c) -> b r c", c=ENB)[:, p : p + cnt, 0:K]
        nc.scalar.activation(
            dst, em[:, 0 : cnt * K].rearrange("b (t k) -> b t k", t=cnt),
            AF.Exp, bias=cst[0:BSH, 0:1],
        )

    etp_cur = [None, None]

    def en_slot(seg, s):
        p = (s - 1) % ENRING
        return en[seg].rearrange("b (r c) -> b r c", c=ENB)[:, p, :]

    def issue_transposes(s):
        """PE transposes of slot s emissions into the 4-slot etp psum block."""
        j = (s - 1) % 4
        if j == 0:
            for g in range(2):
                etp_cur[g] = etp_pool[g].tile(
                    [KP, 4 * GW], BF16, tag="etp", name=f"etpb{g}"
                )
        for g, l in ((sg2 // 4, sg2 % 4) for sg2 in range(S)):
            seg = 4 * g + l
            if s > _last_em_slot(seg):
                continue
            nc.tensor.transpose(
                etp_cur[g][:, j * GW + l * K : j * GW + (l + 1) * K],
                en_slot(seg, s), ident,
            )

    def issue_copy(blk):
        """Batch copy of a completed 4-slot etp block PSUM -> etr SBUF.
        Group 0 on ACT, group 1 on DVE (2x bf16 mode) to balance load."""
        q = (4 * blk) % TRRING
        nc.scalar.copy(etr[0][:, q * GW : (q + 4) * GW], etp_cur[0][:])
        nc.vector.tensor_copy(etr[1][:, q * GW : (q + 4) * GW], etp_cur[1][:])

    def capture_csum(seg, wcs, eng):
        """DMA colsum rows of completed chain windows into the global Z
        table: csum[row, tw*64+b] with row = global tau window."""
        g, l = seg // 4, seg % 4
        valid = []
        for wc in wcs:
            if seg == 0 and wc > SEGLEN // W - 1:
                continue
            if seg > 0 and wc < NMIX // W:
                continue
            valid.append(wc)
        if not valid:
            return
        w0, w1 = valid[0], valid[-1]
        row = w0 if seg == 0 else W * seg + w0 - NMIX // W
        src = wh[g][K : K + 1, :].rearrange("p (s c) -> p s c", c=GW)[
            :, W * w0 + 1 : W * w1 + W + 1, l * K : (l + 1) * K
        ]
        eng.dma_start(csum[row : row + len(valid), :], src)

    # ---- input DMAs first: the DMA queues stream while engines set up ----
    DMAQ = (nc.sync, nc.scalar, nc.gpsimd)
    tr_t = fin_pool.tile([K, K], F32, tag="tr")
    nc.sync.dma_start(tr_t[:], trans_d)
    a0_t = fin_pool.tile([K, 1], F32, tag="a0")
    nc.sync.dma_start(a0_t[:], alpha0_d)
    chunks = {}
    for seg in range(S):
        chunks[(seg, 0)] = load_chunk(seg, 0, eng=DMAQ[seg % 3])
    e0_t = fin_pool.tile([BSH, K], F32, tag="e0")
    nc.scalar.dma_start(e0_t[:], emits_d[0])
    # mask straight load: maskw[win, tw*64+b] = mask[8*win+tw, b]
    nc.sync.dma_start(mk_u8[:, :], mask_d.rearrange("(w t) b -> w (t b)", t=W))
    for w in (1, 2):
        for seg in range(S):
            chunks[(seg, w)] = load_chunk(seg, w, eng=DMAQ[(seg + w) % 3])

    # ---- engine-parallel setup ----
    nc.vector.memset(cst[:, 0:1], 0.0)
    nc.vector.memset(cst[:, 1:2], -DELTA)
    nc.vector.memset(ets[:, K : K + 1], 1.0)
    nc.vector.memset(expal[K : K + 1, :], 1.0)
    # initial states at slot 0: ones for guessed segments 1..7
    nc.vector.memset(wh[0][:, K:GW], 1.0)
    nc.vector.memset(wh[1][:, 0:GW], 1.0)
    make_identity(nc, ident)
    # ones columns of the padded emission rings (become the etr ones rows)
    for s in range(S):
        nc.gpsimd.memset(
            en[s].rearrange("b (r c) -> b r c", c=ENB)[:, :, K : K + 1], 1.0
        )
    nc.gpsimd.iota(iota_i[:, :], pattern=[[1, W], [0, BSH]], base=0,
                   channel_multiplier=W)
    nc.vector.tensor_copy(iotaw[:, :], iota_i[:, :])
    nc.gpsimd.memset(ones_c[:, :], 1.0)
    nc.gpsimd.memset(ones8[:, :], 1.0)
    # sel[w, r] = 1.0 if w >= 8r  (engine writes must start at 32-aligned
    # partitions, so build it from an iota + compare instead of memsets)
    sel_i = nc.alloc_sbuf_tensor("sel_i", [K, S], I32).ap()
    nc.gpsimd.iota(sel_i[:, :], pattern=[[-W, S]], base=0, channel_multiplier=1)
    sel_f = nc.alloc_sbuf_tensor("sel_f", [K, S], F32).ap()
    nc.vector.tensor_copy(sel_f[:, :], sel_i[:, :])
    nc.vector.tensor_scalar(sel[:, :], sel_f[:, :], 0.0, None,
                            op0=mybir.AluOpType.is_ge)
    nc.gpsimd.memset(anch[:, :], 1.0)
    nc.gpsimd.memset(prev[:, :], 1.0)

    # ---- activations + window 0/1 staging ----
    nc.scalar.activation(ets[:, 0:K], tr_t[:], AF.Exp, bias=cst[0:K, 1:2])
    nc.scalar.activation(expal[0:K, :], a0_t[:], AF.Exp, bias=cst[0:K, 0:1])
    e0_en = fin_pool.tile([BSH, KP], BF16, tag="e0en")
    nc.vector.memset(e0_en[:, K : K + 1], 1.0)
    nc.scalar.activation(e0_en[:, 0:K], e0_t[:], AF.Exp, bias=cst[0:BSH, 0:1])
    for seg in range(S):
        exp_chunk(seg, 0, chunks.pop((seg, 0)))
    # seg 0 initial state w_0 = exp(alpha_0) * e_0 (transposed)
    e0_ps = etp_pool[0].tile([KP, K], BF16, tag="etp", name="e0ps")
    nc.tensor.transpose(e0_ps[:], e0_en[:], ident)
    nc.vector.tensor_scalar(
        wh[0][:, 0:K], e0_ps[:], expal, None, op0=MULT
    )
    for s in range(1, 5):
        issue_transposes(s)
    issue_copy(0)
    for s in range(5, W + 1):
        issue_transposes(s)
    issue_copy(1)
    for seg in range(S):
        exp_chunk(seg, 1, chunks.pop((seg, 1)))

    # mask-derived final-combine pieces (off the critical path)
    nc.vector.tensor_copy(maskw[:, :], mk_u8[:, :])
    mwin = fin_pool.tile([K, K], F32, tag="mwin")
    nc.vector.tensor_reduce(
        mwin[:], maskw[:, :].rearrange("p (t b) -> p b t", t=W), axis=AX, op=ADD
    )
    prodt = fin_pool.tile([K, T], F32, tag="prodt")
    nc.vector.tensor_tensor(prodt[:], maskw[:, :], iotaw[:, :], op=MULT)
    redt = fin_pool.tile([K, K], F32, tag="redt")
    nc.vector.tensor_reduce(
        redt[:], prodt[:].rearrange("p (t b) -> p b t", t=W), axis=AX, op=ADD
    )
    cm_ps = st_pool[0].tile([S, BSH], F32, tag="st")
    nc.tensor.matmul(cm_ps[:], sel[:, :], mwin[:], start=True, stop=True)
    cm_sb = fin_pool.tile([S, BSH], F32, tag="cm")
    nc.vector.tensor_copy(cm_sb[:], cm_ps[:])
    redt_ps = st_pool[1].tile([1, BSH], F32, tag="st")
    nc.tensor.matmul(redt_ps[:], ones_c, redt[:], start=True, stop=True)
    redtr = fin_pool.tile([1, BSH], F32, tag="redtr")
    nc.vector.tensor_copy(redtr[:], redt_ps[:])

    # ---- main slot loop ----
    # Staging spread one segment per slot: at slot sg (k = (sg-1)%8,
    # w = (sg-1)//8) issue segment k's chunk DMA for window w+3, its exp and
    # XBAR transpose for window w+2, and (on even windows) its 2-window
    # colsum-capture DMA.
    for sg in range(1, NSLOT + 1):
        k = (sg - 1) % W
        w = (sg - 1) // W
        # chain MMs first so a stalled staging op can't block them in the
        # 4-deep PE wait queue
        st_t = [None, None]
        for g in range(2):
            lo = K if (g == 0 and sg > SEGLEN) else 0
            st_t[g] = st_pool[g].tile([KP, GW], F32, tag="st",
                                      name=f"stt{g}")
            nc.tensor.matmul(
                st_t[g][:, lo:GW], ets[:, :],
                wh[g][0:K, (sg - 1) * GW + lo : sg * GW],
                start=True, stop=True,
            )
        q = (sg - 1) % TRRING
        for g in range(2):
            lo = K if (g == 0 and sg > SEGLEN) else 0
            hi = 3 * K if (g == 1 and sg == NSLOT) else GW
            nc.vector.tensor_tensor(
                wh[g][:, sg * GW + lo : sg * GW + hi],
                st_t[g][:, lo:hi],
                etr[g][0:KP, q * GW + lo : q * GW + hi],
                op=MULT,
            )
        if w + 3 < NWINS:
            chunks[(k, w + 3)] = load_chunk(k, w + 3)
        if k < 4:
            # exps compressed into the window's first 4 slots (2 per slot)
            # so transposes 8+ slots later never wait on ACT
            for seg in (2 * k, 2 * k + 1):
                em = chunks.pop((seg, w + 2), None)
                if em is not None:
                    exp_chunk(seg, w + 2, em)
        if w >= 2 and w % 2 == 0:
            capture_csum(k, (w - 2, w - 1), nc.gpsimd)
        s_look = sg + W
        if s_look <= NSLOT:
            issue_transposes(s_look)
            if s_look % 4 == 0:
                issue_copy(s_look // 4 - 1)
        if sg == NSLOT:
            # segment 7 slot 72 (t=512): only the colsum row is needed
            nc.vector.tensor_copy(
                wh[1][K : K + 1, sg * GW + 3 * K : (sg + 1) * GW],
                st_t[1][K : K + 1, 3 * K : GW],
            )
        if sg == NMIX + 2:
            # mixing anchors: Zg_s at tau = 64s-1 = row 64 of slot NMIX
            o = NMIX * GW
            nc.gpsimd.dma_start(anch[1:4, :], wh[0][K : K + 1, o + K : o + GW])
            nc.gpsimd.dma_start(anch[4:8, :], wh[1][K : K + 1, o : o + GW])

    # prev anchors: Z_{s-1} at tau = 64s-1 (seg 0: slot 64; segs 1..6: slot 72)
    nc.gpsimd.dma_start(prev[1:2, :], wh[0][K : K + 1, SEGLEN * GW : SEGLEN * GW + K])
    o = NSLOT * GW
    nc.gpsimd.dma_start(prev[2:5, :], wh[0][K : K + 1, o + K : o + GW])
    nc.gpsimd.dma_start(prev[5:8, :], wh[1][K : K + 1, o : o + 3 * K])

    # last chain window's colsum captures (the loop covered windows 0..7)
    for seg in range(S):
        capture_csum(seg, (NWINS - 1,), DMAQ[seg % 3])

    # ---- final combine ----
    prodz = fin_pool.tile([K, T], F32, tag="prodz")
    nc.vector.tensor_tensor(prodz[:], csum[:, :], maskw[:, :], op=MULT)
    redz = fin_pool.tile([K, K], F32, tag="redz")
    nc.vector.tensor_reduce(
        redz[:], prodz[:].rearrange("p (t b) -> p b t", t=W), axis=AX, op=ADD
    )
    z_ps = st_pool[0].tile([1, BSH], F32, tag="st")
    nc.tensor.matmul(z_ps[:], ones_c, redz[:], start=True, stop=True)
    lnz = fin_pool.tile([1, BSH], F32, tag="lnz")
    nc.scalar.activation(lnz[:], z_ps[:], AF.Ln, bias=cst[0:1, 0:1])

    # segment corrections: delta_s = ln(prev_s) - ln(anch_s), cumulated via
    # the win>=8r selector matmul (cm_sb[r,b] = [tau_b >= 64r])
    lnp = fin_pool.tile([S, BSH], F32, tag="lnp")
    nc.scalar.activation(lnp[:], prev[:, :], AF.Ln, bias=cst[0:S, 0:1])
    lna = fin_pool.tile([S, BSH], F32, tag="lna")
    nc.scalar.activation(lna[:], anch[:, :], AF.Ln, bias=cst[0:S, 0:1])
    dls = fin_pool.tile([S, BSH], F32, tag="dls")
    nc.vector.tensor_tensor(dls[:], lnp[:], lna[:], op=SUB)
    corr_in = fin_pool.tile([S, BSH], F32, tag="corr_in")
    nc.vector.tensor_tensor(corr_in[:], dls[:], cm_sb[:], op=MULT)
    corr_ps = st_pool[1].tile([1, BSH], F32, tag="st")
    nc.tensor.matmul(corr_ps[:], ones8, corr_in[:], start=True, stop=True)

    res0 = fin_pool.tile([1, BSH], F32, tag="res0")
    nc.vector.scalar_tensor_tensor(
        res0[:], redtr[:], DELTA, lnz[:], op0=MULT, op1=ADD
    )
    res = fin_pool.tile([1, BSH], F32, tag="res")
    nc.vector.tensor_tensor(res[:], res0[:], corr_ps[:], op=ADD)
    nc.sync.dma_start(out_d, res[:])


_NC_CACHE = None


def _get_nc():
    global _NC_CACHE
    if _NC_CACHE is None:
        _NC_CACHE = _build_crf_nc()
    return _NC_CACHE


def _make_in_maps(np_inputs):
    emits = np.asarray(np_inputs["emits"], dtype=np.float32)
    mask_u8 = np.asarray(np_inputs["mask"]).astype(np.uint8)
    transitions = np.asarray(np_inputs["transitions"], dtype=np.float32)
    alpha_0 = np.asarray(np_inputs["alpha_0"], dtype=np.float32)
    in_maps = []
    for c in range(NCORES):
        sl = slice(c * BSH, (c + 1) * BSH)
        in_maps.append(
            {
                "emits": np.ascontiguousarray(emits[:, sl, :]),
                "maskb": np.ascontiguousarray(mask_u8[:, sl]),
                "transitions": transitions,
                "alpha_0": alpha_0,
            }
        )
    return in_maps


def kernel(emits, mask, transitions, alpha_0):
    nc = _get_nc()
    in_maps = _make_in_maps(
        {"emits": emits, "mask": mask, "transitions": transitions,
         "alpha_0": alpha_0}
    )
    res = run_bass_kernel_spmd(nc, in_maps, core_ids=list(range(NCORES)))
    total = np.float64(0.0)
    for r in res.results:
        total += np.asarray(r["out_row"], dtype=np.float64).sum()
    return np.float32(total)


# revision 3
# speedup vs baseline: 1.0003x; 1.0003x over previous
"""CRF forward (logsumexp over paths) loss kernel for Trainium2, 8 NeuronCores.

Math
----
reference:  fv0 = alpha_0^T + emits[0]                       [B, K]
            fv_t[b,j] = logsumexp_i(fv_{t-1}[b,i] + trans[i,j]) + emit_t[b,j]
            alpha_z = sum_b logsumexp_k( fv_{tau_b}[b,:] )   (tau = one-hot mask step)

Exp-space recurrence: with ETs[i,j] = exp(trans[i,j] - DELTA) (plus a 65th
ones column) and e_t[k,b] = exp(emit_t[b,k]) transposed (plus a ones row),
the state w_t[k,b] = exp(fv_t[k,b] - DELTA*t) obeys

    w_t = (ETs^T w_{t-1}) * e_t        (one matmul + one elementwise mul)

No renormalization is needed: for DELTA = 5.125 the state stays within
~e^-20..e^+10 over all 512 steps (bf16 shares fp32's 8-bit exponent).  Row 64
of each matmul output is colsum(w_{t-1}) = Z_{t-1} (the logsumexp of fv_{t-1}
up to the known DELTA*t offset), captured for free through the ets ones
column and the emission ones row.

Time-parallel segmentation: the serial chain latency (~0.9us per step group:
PE->PSUM latency + DVE round trip) would force 512 x ~0.5us serial steps.
Products of positive matrices forget their initial condition (Birkhoff
contraction; mixing completes in <8 steps here), so time is cut into S=8
segments of 64 steps.  Segment s>=1 starts NMIX=8 steps early from an
all-ones state; after mixing its state equals the true state up to a
per-batch-column scalar.  The scalar is recovered by anchoring: both
segment s and segment s-1 compute Z at tau = 64s-1, and
log c_s = log Zg_s - log Z_{s-1} there.  Cascaded corrections are applied
via a segment-selector matmul in the final combine.

All 8 segments run concurrently in lockstep slots, grouped 4+4: per group
per slot ONE fused matmul (rhs = 4 segments' states, 256 cols) and ONE DVE
tensor_tensor [65,256] (amortizing the fixed ~125ns PSUM-access cost 4 ways).
Emissions are exp'ed on ACT into a PADDED natural-layout ring (128-col slot
blocks: 64 emission cols, a ones col at 64) and transposed by the DMA XBAR
engine (hardware 16x128-tile transpose, one DMA per segment-window) straight
into SBUF - the ones column lands as the ones ROW at partition 64, so the
tensor engine does nothing but the chain matmuls.  The full state history
stays in SBUF; colsum rows are DMA-captured per window into the Z table and
the mask-select reduction runs at the end.

Sharding: batch B=512 split across 8 cores (64 per core); transitions/alpha_0
replicated; final alpha_z = host sum of the 8 per-core [1,64] row outputs.
"""

import os
import sys

for _p in ("/opt/trn_rl_repo", "/root/.axon_site/_ro/trn_rl_repo"):
    if os.path.isdir(_p) and _p not in sys.path:
        sys.path.insert(0, _p)

from contextlib import ExitStack

import numpy as np

import concourse.bass as bass
import concourse.mybir as mybir
import concourse.tile as tile
from concourse.bass_utils import run_bass_kernel_spmd
from concourse.masks import make_identity

# The walrus build in this container rejects instructions carrying more than
# one sync-wait command ("Too many sync wait commands" in setupSyncWait).
# Tile freely emits multi-wait instructions, so split the extras onto
# preceding same-engine no-ops at commit time (engine queues execute
# in-order, so the semantics are identical).
_ORIG_COMMIT = tile.TileContext._commit_instruction


def _single_wait_commit(self, inst, lazy_reg_writes=True):
    si = getattr(inst, "sync_info", None)
    if (
        si is not None
        and si.on_wait
        and len(si.on_wait) > 1
        and inst.engine != mybir.EngineType.Unassigned
    ):
        waits = list(si.on_wait)
        eng = self.nc.engines[inst.engine]
        for w in waits[:-1]:
            n = eng.nop(nofuse=True)
            n.ins.sync_info = mybir.SyncInfo(on_wait=[w], on_update=[])
        inst.sync_info = mybir.SyncInfo(
            on_wait=[waits[-1]], on_update=list(si.on_update or [])
        )
    _ORIG_COMMIT(self, inst, lazy_reg_writes)


tile.TileContext._commit_instruction = _single_wait_commit

T, B, K = 512, 512, 64
NCORES = 8
BSH = B // NCORES          # 64 batch elements per core
S = 8                      # time segments
SEGLEN = T // S            # 64 output steps per segment
NMIX = 8                   # warm-up (mixing) steps for guessed segments
NSLOT = SEGLEN + NMIX      # 72 chain slots per segment
W = 8                      # slots per emission-staging window
NWINS = NSLOT // W         # 9 windows
GW = 4 * K                 # 256: group width (4 segments x 64 cols)
KP = K + 1                 # 65 rows: state + colsum row
ENB = K + 1                # 65: natural-layout slot block (emis | ones col)
ENRING = 3 * W             # 24-slot natural-emission ring
TRRING = 3 * W             # 24-slot transposed-emission ring
DELTA = 5.125              # per-step log-space offset folded into ETs
F32 = mybir.dt.float32
BF16 = mybir.dt.bfloat16
U8 = mybir.dt.uint8
I32 = mybir.dt.int32
MULT = mybir.AluOpType.mult
ADD = mybir.AluOpType.add
SUB = mybir.AluOpType.subtract
AX = mybir.AxisListType.X
AF = mybir.ActivationFunctionType


def _seg_t0(seg):  # t value of chain slot 0 (initial state)
    return 0 if seg == 0 else SEGLEN * seg - NMIX


def _last_em_slot(seg):  # last slot with a real emission
    if seg == 0:
        return SEGLEN          # seg 0 produces tau 0..63 from slots 1..64
    if seg == S - 1:
        return NSLOT - 1       # t(NSLOT) = 512 has no emission
    return NSLOT


def _build_crf_nc() -> bass.Bass:
    nc = bass.Bass(trn_type="TRN2", target_bir_lowering=False, debug=False)

    emits_d = nc.dram_tensor("emits", [T, BSH, K], F32, kind="ExternalInput").ap()
    mask_d = nc.dram_tensor("maskb", [T, BSH], U8, kind="ExternalInput").ap()
    trans_d = nc.dram_tensor("transitions", [K, K], F32, kind="ExternalInput").ap()
    alpha0_d = nc.dram_tensor("alpha_0", [K, 1], F32, kind="ExternalInput").ap()
    out_d = nc.dram_tensor("out_row", [1, BSH], F32, kind="ExternalOutput").ap()

    with tile.TileContext(nc) as tc:
        with ExitStack() as ctx:
            _crf_body(ctx, tc, emits_d, mask_d, trans_d, alpha0_d, out_d)
    _split_remaining_multiwaits(nc)
    return nc


def _split_remaining_multiwaits(nc):
    """Split multi-wait instructions added outside the commit path (e.g. the
    end-of-kernel drain/barrier) onto preceding same-engine no-ops."""
    for blk in nc.m.functions[0].blocks:
        il = blk.instructions
        idx = 0
        while idx < len(il):
            inst = il[idx]
            si = inst.sync_info
            if si is not None and si.on_wait and len(si.on_wait) > 1:
                waits = list(si.on_wait)
                for j, w in enumerate(waits[:-1]):
                    n = mybir.InstNoOp(
                        name=f"I-swx-{inst.name}-{j}", ins=[], outs=[]
                    )
                    n.engine = inst.engine
                    n.sync_info = mybir.SyncInfo(on_wait=[w], on_update=[])
                    nc.register_instruction(n, overwrite=True)
                    il.insert(idx, n)
                    idx += 1
                inst.sync_info = mybir.SyncInfo(
                    on_wait=[waits[-1]], on_update=list(si.on_update or [])
                )
            idx += 1


def _crf_body(ctx, tc, emits_d, mask_d, trans_d, alpha0_d, out_d):
    nc = tc.nc

    # ---- long-lived SBUF state ----
    ets = nc.alloc_sbuf_tensor("ets", [K, KP], BF16).ap()     # exp(trans-d)|1
    expal = nc.alloc_sbuf_tensor("expal", [KP, 1], F32).ap()  # exp(alpha_0)|1
    # per-group state history: slot sigma block = 4 segments x 64 cols,
    # row 64 of slot sigma = colsum of the slot sigma-1 state
    wh = [
        nc.alloc_sbuf_tensor(f"wh{g}", [KP, (NSLOT + 1) * GW], BF16).ap()
        for g in range(2)
    ]
    # transposed-emission rings: slot block = [65, 4 segs x 64], row 64 = ones
    etr = [
        nc.alloc_sbuf_tensor(f"etr{g}", [KP, TRRING * GW], BF16).ap()
        for g in range(2)
    ]
    # post-exp emissions, natural layout: slot block = [64, 65 (emis | ones)]
    en = [
        nc.alloc_sbuf_tensor(f"en{s}", [BSH, ENRING * ENB], BF16).ap()
        for s in range(S)
    ]
    ident = nc.alloc_sbuf_tensor("ident", [BSH, BSH], BF16).ap()
    maskw = nc.alloc_sbuf_tensor("maskw", [K, T], F32).ap()   # [win, tw*64+b]
    mk_u8 = nc.alloc_sbuf_tensor("mk_u8", [K, T], U8).ap()
    iota_i = nc.alloc_sbuf_tensor("iota_i", [K, T], I32).ap()
    iotaw = nc.alloc_sbuf_tensor("iotaw", [K, T], F32).ap()   # t at slot pos
    csum = nc.alloc_sbuf_tensor("csum", [K, T], BF16).ap()    # Z_tau table
    sel = nc.alloc_sbuf_tensor("sel", [K, S], F32).ap()       # win>=8r matrix
    ones_c = nc.alloc_sbuf_tensor("ones_c", [K, 1], F32).ap()
    ones8 = nc.alloc_sbuf_tensor("ones8", [S, 1], F32).ap()
    anch = nc.alloc_sbuf_tensor("anch", [S, BSH], BF16).ap()  # Zg_s anchors
    prev = nc.alloc_sbuf_tensor("prev", [S, BSH], BF16).ap()  # Z_{s-1} anchors
    cst = nc.alloc_sbuf_tensor("cst", [K, 2], F32).ap()       # bias constants

    # ---- pools ----
    em_pool = ctx.enter_context(tc.tile_pool(name="em", bufs=18))
    fin_pool = ctx.enter_context(tc.tile_pool(name="fin", bufs=1))
    # single shared pools with 3 bufs: allocation order alternates groups, so
    # the buffer-reuse dependency bounds group desync to ~1.5 slots and keeps
    # the static schedule honest
    st_shared = ctx.enter_context(tc.tile_pool(name="st", bufs=3, space="PSUM"))
    st_pool = [st_shared, st_shared]
    etp_shared = ctx.enter_context(tc.tile_pool(name="etp", bufs=3, space="PSUM"))
    etp_pool = [etp_shared, etp_shared]

    # ---- emission staging helpers ----
    def load_chunk(seg, w, eng=None):
        """DMA the raw fp32 emissions for (segment, window) into an em tile."""
        lo, hi = W * w + 1, min(W * w + W, _last_em_slot(seg))
        if lo > hi:
            return None
        t0 = _seg_t0(seg)
        cnt = hi - lo + 1
        em = em_pool.tile([BSH, W * K], F32, tag="em", name="em")
        (eng or nc.sync).dma_start(
            em[:, 0 : cnt * K].rearrange("b (t k) -> b t k", t=cnt),
            emits_d[t0 + lo : t0 + hi + 1].rearrange("t b k -> b t k"),
        )
        return em

    def exp_chunk(seg, w, em):
        """exp -> bf16 into the padded natural-layout emission ring."""
        lo, hi = W * w + 1, min(W * w + W, _last_em_slot(seg))
        if lo > hi:
            return
        cnt = hi - lo + 1
        p = (lo - 1) % ENRING
        dst = en[seg].rearrange("b (r c) -> b r c", c=ENB)[:, p : p + cnt, 0:K]
        nc.scalar.activation(
            dst, em[:, 0 : cnt * K].rearrange("b (t k) -> b t k", t=cnt),
            AF.Exp, bias=cst[0:BSH, 0:1],
        )

    etp_cur = [None, None]

    def en_slot(seg, s):
        p = (s - 1) % ENRING
        return en[seg].rearrange("b (r c) -> b r c", c=ENB)[:, p, :]

    def issue_transposes(s):
        """PE transposes of slot s emissions into the 4-slot etp psum block."""
        j = (s - 1) % 4
        if j == 0:
            for g in range(2):
                etp_cur[g] = etp_pool[g].tile(
                    [KP, 4 * GW], BF16, tag="etp", name=f"etpb{g}"
                )
        for g, l in ((sg2 // 4, sg2 % 4) for sg2 in range(S)):
            seg = 4 * g + l
            if s > _last_em_slot(seg):
                continue
            nc.tensor.transpose(
                etp_cur[g][:, j * GW + l * K : j * GW + (l + 1) * K],
                en_slot(seg, s), ident,
            )

    def issue_copy(blk):
        """Batch copy of a completed 4-slot etp block PSUM -> etr SBUF.
        Group 0 on ACT, group 1 on DVE (2x bf16 mode) to balance load."""
        q = (4 * blk) % TRRING
        nc.scalar.copy(etr[0][:, q * GW : (q + 4) * GW], etp_cur[0][:])
        nc.vector.tensor_copy(etr[1][:, q * GW : (q + 4) * GW], etp_cur[1][:])

    def capture_csum(seg, wcs, eng):
        """DMA colsum rows of completed chain windows into the global Z
        table: csum[row, tw*64+b] with row = global tau window."""
        g, l = seg // 4, seg % 4
        valid = []
        for wc in wcs:
            if seg == 0 and wc > SEGLEN // W - 1:
                continue
            if seg > 0 and wc < NMIX // W:
                continue
            valid.append(wc)
        if not valid:
            return
        w0, w1 = valid[0], valid[-1]
        row = w0 if seg == 0 else W * seg + w0 - NMIX // W
        src = wh[g][K : K + 1, :].rearrange("p (s c) -> p s c", c=GW)[
            :, W * w0 + 1 : W * w1 + W + 1, l * K : (l + 1) * K
        ]
        eng.dma_start(csum[row : row + len(valid), :], src)

    # ---- input DMAs first: the DMA queues stream while engines set up ----
    DMAQ = (nc.sync, nc.scalar, nc.gpsimd)
    tr_t = fin_pool.tile([K, K], F32, tag="tr")
    nc.sync.dma_start(tr_t[:], trans_d)
    a0_t = fin_pool.tile([K, 1], F32, tag="a0")
    nc.sync.dma_start(a0_t[:], alpha0_d)
    chunks = {}
    for seg in range(S):
        chunks[(seg, 0)] = load_chunk(seg, 0, eng=DMAQ[seg % 3])
    e0_t = fin_pool.tile([BSH, K], F32, tag="e0")
    nc.scalar.dma_start(e0_t[:], emits_d[0])
    # mask straight load: maskw[win, tw*64+b] = mask[8*win+tw, b]
    nc.sync.dma_start(mk_u8[:, :], mask_d.rearrange("(w t) b -> w (t b)", t=W))
    for w in (1, 2):
        for seg in range(S):
            chunks[(seg, w)] = load_chunk(seg, w, eng=DMAQ[(seg + w) % 3])

    # ---- engine-parallel setup ----
    nc.vector.memset(cst[:, 0:1], 0.0)
    nc.vector.memset(cst[:, 1:2], -DELTA)
    nc.vector.memset(ets[:, K : K + 1], 1.0)
    nc.vector.memset(expal[K : K + 1, :], 1.0)
    # initial states at slot 0: ones for guessed segments 1..7
    nc.vector.memset(wh[0][:, K:GW], 1.0)
    nc.vector.memset(wh[1][:, 0:GW], 1.0)
    make_identity(nc, ident)
    # ones columns of the padded emission rings (become the etr ones rows)
    for s in range(S):
        nc.gpsimd.memset(
            en[s].rearrange("b (r c) -> b r c", c=ENB)[:, :, K : K + 1], 1.0
        )
    nc.gpsimd.iota(iota_i[:, :], pattern=[[1, W], [0, BSH]], base=0,
                   channel_multiplier=W)
    nc.vector.tensor_copy(iotaw[:, :], iota_i[:, :])
    nc.gpsimd.memset(ones_c[:, :], 1.0)
    nc.gpsimd.memset(ones8[:, :], 1.0)
    # sel[w, r] = 1.0 if w >= 8r  (engine writes must start at 32-aligned
    # partitions, so build it from an iota + compare instead of memsets)
    sel_i = nc.alloc_sbuf_tensor("sel_i", [K, S], I32).ap()
    nc.gpsimd.iota(sel_i[:, :], pattern=[[-W, S]], base=0, channel_multiplier=1)
    sel_f = nc.alloc_sbuf_tensor("sel_f", [K, S], F32).ap()
    nc.vector.tensor_copy(sel_f[:, :], sel_i[:, :])
    nc.vector.tensor_scalar(sel[:, :], sel_f[:, :], 0.0, None,
                            op0=mybir.AluOpType.is_ge)
    nc.gpsimd.memset(anch[:, :], 1.0)
    nc.gpsimd.memset(prev[:, :], 1.0)

    # ---- activations + window 0/1 staging ----
    nc.scalar.activation(ets[:, 0:K], tr_t[:], AF.Exp, bias=cst[0:K, 1:2])
    nc.scalar.activation(expal[0:K, :], a0_t[:], AF.Exp, bias=cst[0:K, 0:1])
    e0_en = fin_pool.tile([BSH, KP], BF16, tag="e0en")
    nc.vector.memset(e0_en[:, K : K + 1], 1.0)
    nc.scalar.activation(e0_en[:, 0:K], e0_t[:], AF.Exp, bias=cst[0:BSH, 0:1])
    for seg in range(S):
        exp_chunk(seg, 0, chunks.pop((seg, 0)))
    # seg 0 initial state w_0 = exp(alpha_0) * e_0 (transposed)
    e0_ps = etp_pool[0].tile([KP, K], BF16, tag="etp", name="e0ps")
    nc.tensor.transpose(e0_ps[:], e0_en[:], ident)
    nc.vector.tensor_scalar(
        wh[0][:, 0:K], e0_ps[:], expal, None, op0=MULT
    )
    for s in range(1, 5):
        issue_transposes(s)
    issue_copy(0)
    for s in range(5, W + 1):
        issue_transposes(s)
    issue_copy(1)
    for seg in range(S):
        exp_chunk(seg, 1, chunks.pop((seg, 1)))

    # mask-derived final-combine pieces (off the critical path)
    nc.vector.tensor_copy(maskw[:, :], mk_u8[:, :])
    mwin = fin_pool.tile([K, K], F32, tag="mwin")
    nc.vector.tensor_reduce(
        mwin[:], maskw[:, :].rearrange("p (t b) -> p b t", t=W), axis=AX, op=ADD
    )
    prodt = fin_pool.tile([K, T], F32, tag="prodt")
    nc.vector.tensor_tensor(prodt[:], maskw[:, :], iotaw[:, :], op=MULT)
    redt = fin_pool.tile([K, K], F32, tag="redt")
    nc.vector.tensor_reduce(
        redt[:], prodt[:].rearrange("p (t b) -> p b t", t=W), axis=AX, op=ADD
    )
    cm_ps = st_pool[0].tile([S, BSH], F32, tag="st")
    nc.tensor.matmul(cm_ps[:], sel[:, :], mwin[:], start=True, stop=True)
    cm_sb = fin_pool.tile([S, BSH], F32, tag="cm")
    nc.vector.tensor_copy(cm_sb[:], cm_ps[:])
    redt_ps = st_pool[1].tile([1, BSH], F32, tag="st")
    nc.tensor.matmul(redt_ps[:], ones_c, redt[:], start=True, stop=True)
    redtr = fin_pool.tile([1, BSH], F32, tag="redtr")
    nc.vector.tensor_copy(redtr[:], redt_ps[:])

    # ---- main slot loop ----
    # Staging spread one segment per slot: at slot sg (k = (sg-1)%8,
    # w = (sg-1)//8) issue segment k's chunk DMA for window w+3, its exp and
    # XBAR transpose for window w+2, and (on even windows) its 2-window
    # colsum-capture DMA.
    for sg in range(1, NSLOT + 1):
        k = (sg - 1) % W
        w = (sg - 1) // W
        # transposes first: their exps completed 8+ slots ago so they are
        # always ready, and ahead of the (waiting) chain MMs they keep the
        # in-order PE queue streaming instead of accumulating into bursts
        s_look = sg + W
        if s_look <= NSLOT:
            issue_transposes(s_look)
        if w + 3 < NWINS:
            chunks[(k, w + 3)] = load_chunk(k, w + 3)
        if k < 4:
            # exps compressed into the window's first 4 slots (2 per slot)
            # so transposes 8+ slots later never wait on ACT
            for seg in (2 * k, 2 * k + 1):
                em = chunks.pop((seg, w + 2), None)
                if em is not None:
                    exp_chunk(seg, w + 2, em)
        if w >= 2 and w % 2 == 0:
            capture_csum(k, (w - 2, w - 1), nc.gpsimd)

        st_t = [None, None]
        for g in range(2):
            lo = K if (g == 0 and sg > SEGLEN) else 0
            st_t[g] = st_pool[g].tile([KP, GW], F32, tag="st",
                                      name=f"stt{g}")
            nc.tensor.matmul(
                st_t[g][:, lo:GW], ets[:, :],
                wh[g][0:K, (sg - 1) * GW + lo : sg * GW],
                start=True, stop=True,
            )
        q = (sg - 1) % TRRING
        for g in range(2):
            lo = K if (g == 0 and sg > SEGLEN) else 0
            hi = 3 * K if (g == 1 and sg == NSLOT) else GW
            nc.vector.tensor_tensor(
                wh[g][:, sg * GW + lo : sg * GW + hi],
                st_t[g][:, lo:hi],
                etr[g][0:KP, q * GW + lo : q * GW + hi],
                op=MULT,
            )
        if s_look <= NSLOT and s_look % 4 == 0:
            issue_copy(s_look // 4 - 1)
        if sg == NSLOT:
            # segment 7 slot 72 (t=512): only the colsum row is needed
            nc.vector.tensor_copy(
                wh[1][K : K + 1, sg * GW + 3 * K : (sg + 1) * GW],
                st_t[1][K : K + 1, 3 * K : GW],
            )
        if sg == NMIX + 2:
            # mixing anchors: Zg_s at tau = 64s-1 = row 64 of slot NMIX
            o = NMIX * GW
            nc.gpsimd.dma_start(anch[1:4, :], wh[0][K : K + 1, o + K : o + GW])
            nc.gpsimd.dma_start(anch[4:8, :], wh[1][K : K + 1, o : o + GW])

    # prev anchors: Z_{s-1} at tau = 64s-1 (seg 0: slot 64; segs 1..6: slot 72)
    nc.gpsimd.dma_start(prev[1:2, :], wh[0][K : K + 1, SEGLEN * GW : SEGLEN * GW + K])
    o = NSLOT * GW
    nc.gpsimd.dma_start(prev[2:5, :], wh[0][K : K + 1, o + K : o + GW])
    nc.gpsimd.dma_start(prev[5:8, :], wh[1][K : K + 1, o : o + 3 * K])

    # last chain window's colsum captures (the loop covered windows 0..7)
    for seg in range(S):
        capture_csum(seg, (NWINS - 1,), DMAQ[seg % 3])

    # ---- final combine ----
    prodz = fin_pool.tile([K, T], F32, tag="prodz")
    nc.vector.tensor_tensor(prodz[:], csum[:, :], maskw[:, :], op=MULT)
    redz = fin_pool.tile([K, K], F32, tag="redz")
    nc.vector.tensor_reduce(
        redz[:], prodz[:].rearrange("p (t b) -> p b t", t=W), axis=AX, op=ADD
    )
    z_ps = st_pool[0].tile([1, BSH], F32, tag="st")
    nc.tensor.matmul(z_ps[:], ones_c, redz[:], start=True, stop=True)
    lnz = fin_pool.tile([1, BSH], F32, tag="lnz")
    nc.scalar.activation(lnz[:], z_ps[:], AF.Ln, bias=cst[0:1, 0:1])

    # segment corrections: delta_s = ln(prev_s) - ln(anch_s), cumulated via
    # the win>=8r selector matmul (cm_sb[r,b] = [tau_b >= 64r])
    lnp = fin_pool.tile([S, BSH], F32, tag="lnp")
    nc.scalar.activation(lnp[:], prev[:, :], AF.Ln, bias=cst[0:S, 0:1])
    lna = fin_pool.tile([S, BSH], F32, tag="lna")
    nc.scalar.activation(lna[:], anch[:, :], AF.Ln, bias=cst[0:S, 0:1])
    dls = fin_pool.tile([S, BSH], F32, tag="dls")
    nc.vector.tensor_tensor(dls[:], lnp[:], lna[:], op=SUB)
    corr_in = fin_pool.tile([S, BSH], F32, tag="corr_in")
    nc.vector.tensor_tensor(corr_in[:], dls[:], cm_sb[:], op=MULT)
    corr_ps = st_pool[1].tile([1, BSH], F32, tag="st")
    nc.tensor.matmul(corr_ps[:], ones8, corr_in[:], start=True, stop=True)

    res0 = fin_pool.tile([1, BSH], F32, tag="res0")
    nc.vector.scalar_tensor_tensor(
        res0[:], redtr[:], DELTA, lnz[:], op0=MULT, op1=ADD
    )
    res = fin_pool.tile([1, BSH], F32, tag="res")
    nc.vector.tensor_tensor(res[:], res0[:], corr_ps[:], op=ADD)
    nc.sync.dma_start(out_d, res[:])


_NC_CACHE = None


def _get_nc():
    global _NC_CACHE
    if _NC_CACHE is None:
        _NC_CACHE = _build_crf_nc()
    return _NC_CACHE


def _make_in_maps(np_inputs):
    emits = np.asarray(np_inputs["emits"], dtype=np.float32)
    mask_u8 = np.asarray(np_inputs["mask"]).astype(np.uint8)
    transitions = np.asarray(np_inputs["transitions"], dtype=np.float32)
    alpha_0 = np.asarray(np_inputs["alpha_0"], dtype=np.float32)
    in_maps = []
    for c in range(NCORES):
        sl = slice(c * BSH, (c + 1) * BSH)
        in_maps.append(
            {
                "emits": np.ascontiguousarray(emits[:, sl, :]),
                "maskb": np.ascontiguousarray(mask_u8[:, sl]),
                "transitions": transitions,
                "alpha_0": alpha_0,
            }
        )
    return in_maps


def kernel(emits, mask, transitions, alpha_0):
    nc = _get_nc()
    in_maps = _make_in_maps(
        {"emits": emits, "mask": mask, "transitions": transitions,
         "alpha_0": alpha_0}
    )
    res = run_bass_kernel_spmd(nc, in_maps, core_ids=list(range(NCORES)))
    total = np.float64(0.0)
    for r in res.results:
        total += np.asarray(r["out_row"], dtype=np.float64).sum()
    return np.float32(total)
